# revision 1
# baseline (speedup 1.0000x reference)
"""ETNN messager layer on 8 Trainium2 NeuronCores.

Edge-parallel, receiver-sharded: host sorts edges by receiver; core k owns
receivers [k*12500,(k+1)*12500) and scatter-adds into its private slice.
Gathers/scatter use indirect_dma_start ([P,1] per-partition offsets, int32).
BN folded into W1 on host. Messages: silu(state @ W1f + b1f),
gate = sigmoid(msg @ W2 + b2). Receivers within a chunk are made distinct by
column-major spreading so CCE-add scatters never collide inside one
instruction; pads go to a dump row.
"""

import numpy as np

import concourse.tile as tile
from concourse import bacc, bass, mybir
from concourse.bass_utils import run_bass_kernel_spmd
from concourse.masks import make_identity

N = 100000
E = 500000
H = 128
INV = 16
NCORES = 8
NLOC = N // NCORES          # 12500 receivers per core
CHUNK = 2048
NCHUNK = 36
SLOTS = NCHUNK * CHUNK      # 73728 slots/core
ST = CHUNK // 128           # 16 subtiles per chunk
BN_EPS = 1e-5

_prog_cache = {}


def _build(b2val: float):
    key = round(b2val, 9)
    if key in _prog_cache:
        return _prog_cache[key]
    nc = bacc.Bacc("TRN2", target_bir_lowering=False, debug=False)
    dt = mybir.dt
    xs = nc.dram_tensor("xs", [N, H], dt.float32, kind="ExternalInput")
    xr = nc.dram_tensor("xr", [NLOC + 1, H], dt.float32, kind="ExternalInput")
    sidx = nc.dram_tensor("sidx", [128, SLOTS // 128], dt.int32, kind="ExternalInput")
    ridx = nc.dram_tensor("ridx", [128, SLOTS // 128], dt.int32, kind="ExternalInput")
    eat = nc.dram_tensor("eat", [INV + 1, SLOTS], dt.float32, kind="ExternalInput")
    wa = nc.dram_tensor("wa", [H, H], dt.float32, kind="ExternalInput")
    wb = nc.dram_tensor("wb", [H, H], dt.float32, kind="ExternalInput")
    wc = nc.dram_tensor("wc", [INV + 1, H], dt.float32, kind="ExternalInput")
    w2b = nc.dram_tensor("w2b", [128, H], dt.float32, kind="ExternalInput")
    out = nc.dram_tensor("out", [NLOC + 1, H], dt.float32, kind="ExternalOutput")

    with tile.TileContext(nc) as tc:
        with tc.tile_pool(name="const", bufs=1) as cp, \
             tc.tile_pool(name="gath", bufs=4) as gp, \
             tc.tile_pool(name="trans", bufs=4) as tp, \
             tc.tile_pool(name="ea", bufs=3) as ep, \
             tc.tile_pool(name="msg", bufs=2) as mp, \
             tc.tile_pool(name="small", bufs=4) as sp, \
             tc.tile_pool(name="psum", bufs=2, space="PSUM") as pp:
            wa_sb = cp.tile([H, H], dt.float32)
            wb_sb = cp.tile([H, H], dt.float32)
            wc_sb = cp.tile([INV + 1, H], dt.float32)
            w2_sb = cp.tile([128, H], dt.float32)
            si_sb = cp.tile([128, SLOTS // 128], dt.int32)
            ri_sb = cp.tile([128, SLOTS // 128], dt.int32)
            ident = cp.tile([128, 128], dt.float32)
            make_identity(nc, ident[:])
            nc.sync.dma_start(out=wa_sb[:], in_=wa[:, :])
            nc.sync.dma_start(out=wb_sb[:], in_=wb[:, :])
            nc.sync.dma_start(out=wc_sb[:], in_=wc[:, :])
            nc.sync.dma_start(out=w2_sb[:], in_=w2b[:, :])
            nc.sync.dma_start(out=si_sb[:], in_=sidx[:, :])
            nc.sync.dma_start(out=ri_sb[:], in_=ridx[:, :])

            for cl in range(NCHUNK):
                ea_sb = ep.tile([INV + 1, CHUNK], dt.float32, tag="ea")
                nc.sync.dma_start(
                    out=ea_sb[:], in_=eat[:, cl * CHUNK : (cl + 1) * CHUNK]
                )
                msg = mp.tile([128, ST, H], dt.float32, tag="m")
                tt = mp.tile([128, ST, H], dt.float32, tag="t")
                ff = mp.tile([128, ST, H], dt.float32, tag="f")
                red = sp.tile([128, ST], dt.float32, tag="red")
                gate = sp.tile([128, ST], dt.float32, tag="gate")
                for j in range(ST):
                    q0 = cl * ST + j  # subtile column in idx tensors
                    js = slice(j * 128, (j + 1) * 128)
                    gs = gp.tile([128, H], dt.float32, tag="gs")
                    gr = gp.tile([128, H], dt.float32, tag="gr")
                    nc.gpsimd.indirect_dma_start(
                        out=gs[:], out_offset=None, in_=xs[:, :],
                        in_offset=bass.IndirectOffsetOnAxis(
                            ap=si_sb[:, q0 : q0 + 1], axis=0),
                    )
                    nc.gpsimd.indirect_dma_start(
                        out=gr[:], out_offset=None, in_=xr[:, :],
                        in_offset=bass.IndirectOffsetOnAxis(
                            ap=ri_sb[:, q0 : q0 + 1], axis=0),
                    )
                    tps = pp.tile([128, H], dt.float32, tag="tps")
                    tpr = pp.tile([128, H], dt.float32, tag="tpr")
                    nc.tensor.transpose(out=tps[:], in_=gs[:], identity=ident[:])
                    nc.tensor.transpose(out=tpr[:], in_=gr[:], identity=ident[:])
                    tss = tp.tile([128, H], dt.float32, tag="tss")
                    trs = tp.tile([128, H], dt.float32, tag="trs")
                    nc.vector.tensor_copy(out=tss[:], in_=tps[:])
                    nc.vector.tensor_copy(out=trs[:], in_=tpr[:])
                    pm = pp.tile([128, H], dt.float32, tag="pm")
                    nc.tensor.matmul(out=pm[:], lhsT=tss[:], rhs=wa_sb[:],
                                     start=True, stop=False)
                    nc.tensor.matmul(out=pm[:], lhsT=trs[:], rhs=wb_sb[:],
                                     start=False, stop=False)
                    nc.tensor.matmul(out=pm[:], lhsT=ea_sb[:, js], rhs=wc_sb[:],
                                     start=False, stop=True)
                    sg = sp.tile([128, H], dt.float32, tag="sg")
                    nc.scalar.activation(
                        out=sg[:], in_=pm[:],
                        func=mybir.ActivationFunctionType.Sigmoid)
                    nc.vector.tensor_tensor(
                        out=msg[:, j, :], in0=pm[:], in1=sg[:],
                        op=mybir.AluOpType.mult)
                    nc.vector.tensor_tensor(
                        out=tt[:, j, :], in0=msg[:, j, :], in1=w2_sb[:],
                        op=mybir.AluOpType.mult)
                nc.vector.tensor_reduce(
                    out=red[:], in_=tt[:, :, :],
                    axis=mybir.AxisListType.X, op=mybir.AluOpType.add)
                nc.scalar.activation(
                    out=gate[:], in_=red[:],
                    func=mybir.ActivationFunctionType.Sigmoid, bias=b2val)
                for j in range(ST):
                    nc.vector.tensor_tensor(
                        out=ff[:, j, :], in0=msg[:, j, :],
                        in1=gate[:, j : j + 1].to_broadcast([128, H]),
                        op=mybir.AluOpType.mult)
                for j in range(ST):
                    q0 = cl * ST + j
                    nc.gpsimd.indirect_dma_start(
                        out=out[:, :],
                        out_offset=bass.IndirectOffsetOnAxis(
                            ap=ri_sb[:, q0 : q0 + 1], axis=0),
                        in_=ff[:, j, :], in_offset=None,
                        compute_op=mybir.AluOpType.add,
                    )
    nc.compile()
    _prog_cache[key] = nc
    return nc


def _host_prep(x_send, x_rec, index, edge_attr, bn_gamma, bn_beta, bn_mean,
               bn_var, W1, b1, W2, b2):
    s = np.asarray(index[0], dtype=np.int64)
    r = np.asarray(index[1], dtype=np.int64)
    ea = np.asarray(edge_attr, dtype=np.float32)

    scale = np.asarray(bn_gamma) / np.sqrt(np.asarray(bn_var) + BN_EPS)
    shift = np.asarray(bn_beta) - np.asarray(bn_mean) * scale
    W1f = (np.asarray(W1) * scale[:, None]).astype(np.float32)
    b1f = (np.asarray(b1) + shift @ np.asarray(W1)).astype(np.float32)

    xs_f = np.asarray(x_send, dtype=np.float32)
    wa = W1f[:H]
    wb = W1f[H : 2 * H]
    wc = np.concatenate([W1f[2 * H :], b1f[None, :]], axis=0)
    w2b = np.broadcast_to(np.asarray(W2, dtype=np.float32).reshape(1, H),
                          (128, H)).copy()
    b2val = float(np.asarray(b2).reshape(-1)[0])

    in_maps = []
    for k in range(NCORES):
        m = (r // NLOC) == k
        sk = s[m]
        rk = (r[m] - k * NLOC).astype(np.int64)
        eak = ea[m]
        n = sk.shape[0]
        assert n <= SLOTS, f"shard overflow {n}"
        xr_loc = np.zeros((NLOC + 1, H), dtype=np.float32)
        xr_loc[:NLOC] = np.asarray(x_rec[k * NLOC : (k + 1) * NLOC],
                                   dtype=np.float32)
        sidx = np.zeros((128, SLOTS // 128), dtype=np.int32)
        ridx = np.full((128, SLOTS // 128), NLOC, dtype=np.int32)
        eat = np.zeros((INV + 1, SLOTS), dtype=np.float32)
        eat[INV, :] = 1.0
        # sort by receiver, spread column-major over chunks so receivers are
        # distinct within each chunk (and each 128-subtile)
        o = np.argsort(rk, kind="stable")
        sk, rk, eak = sk[o], rk[o], eak[o]
        i = np.arange(n)
        c = i % NCHUNK
        q = i // NCHUNK          # slot within chunk, < 2048
        col = c * ST + q // 128  # subtile column
        row = q % 128            # partition
        sidx[row, col] = sk.astype(np.int32)
        ridx[row, col] = rk.astype(np.int32)
        eat[:INV, c * CHUNK + q] = eak.T
        in_maps.append({
            "xs": xs_f, "xr": xr_loc, "sidx": sidx, "ridx": ridx,
            "eat": eat, "wa": wa, "wb": wb, "wc": wc, "w2b": w2b,
        })
    return in_maps, b2val


def kernel(**inputs) -> np.ndarray:
    in_maps, b2val = _host_prep(**inputs)
    nc = _build(b2val)
    res = run_bass_kernel_spmd(nc, in_maps, core_ids=list(range(NCORES)))
    return np.concatenate(
        [res.results[k]["out"][:NLOC] for k in range(NCORES)], axis=0
    ).astype(np.float32)



# revision 9
# speedup vs baseline: 2.9859x; 2.9859x over previous
"""ETNN messager layer on 8 Trainium2 NeuronCores — v2.

Receiver-sharded, window-batched. Core k owns receivers [k*12500, (k+1)*12500),
split into 98 windows of 128 receivers; each window gets S=6 subtiles of 128
edge slots (704+ edges/window never observed; S adapts upward if needed).
Per group of 7 windows: two transposing dma_gathers (bf16) pull x_send /
x_rec rows into [H, edges] layout, bf16 matmuls with BN-folded W1 produce
messages, gate = sigmoid(msg @ W2 + b2), and a gated one-hot membership
matmul segment-sums the window in PSUM. Output is written with plain
sequential DMA — no indirect scatter. Sender tables are compacted per
section on host so gather indices fit int16.
"""

import numpy as np
from ml_dtypes import bfloat16

import concourse.tile as tile
from concourse import bacc, bass, mybir
from concourse.bass_utils import run_bass_kernel_spmd

N = 100000
E = 500000
H = 128
INV = 16
NCORES = 8
NLOC = N // NCORES            # 12500 receivers per core
W = 98                        # 128-receiver windows per core (98*128 = 12544)
G = 7                         # windows per group
NG = W // G                   # 14 groups
XR_ROWS = W * 128             # receiver table rows (< 32768 so int16 works)

_prog_cache = {}


def _build(S: int, splits: tuple):
    """S = subtiles per window; splits = per-section group counts for the
    compacted sender tables (e.g. (7, 7))."""
    key = (S, splits)
    if key in _prog_cache:
        return _prog_cache[key]

    GS = G * S                # subtiles per group
    SLOT_G = GS * 128         # edge slots per group
    SLOTS = NG * SLOT_G       # edge slots per core
    ICOL_G = 2 * SLOT_G // 16  # idx columns per group (xs then xr)
    NB = 6                    # subtiles per silu batch
    assert GS % NB == 0

    nc = bacc.Bacc("TRN2", target_bir_lowering=False, debug=False)
    dt = mybir.dt
    xsts = [
        nc.dram_tensor(f"xst{i}", [32768, H], dt.bfloat16, kind="ExternalInput")
        for i in range(len(splits))
    ]
    xrt = nc.dram_tensor("xrt", [XR_ROWS, H], dt.bfloat16, kind="ExternalInput")
    idxt = nc.dram_tensor("idxt", [128, NG * ICOL_G], dt.int16,
                          kind="ExternalInput")
    eat = nc.dram_tensor("eat", [INV + 1, SLOTS], dt.bfloat16,
                         kind="ExternalInput")
    rlt = nc.dram_tensor("rlt", [128, NG * GS], dt.bfloat16,
                         kind="ExternalInput")
    wa = nc.dram_tensor("wa", [H, H], dt.bfloat16, kind="ExternalInput")
    wb = nc.dram_tensor("wb", [H, H], dt.bfloat16, kind="ExternalInput")
    wc = nc.dram_tensor("wc", [INV + 1, H], dt.bfloat16, kind="ExternalInput")
    w2b = nc.dram_tensor("w2b", [128, H], dt.bfloat16, kind="ExternalInput")
    b2t = nc.dram_tensor("b2t", [128, 1], dt.float32, kind="ExternalInput")
    iot = nc.dram_tensor("iot", [128, 128], dt.bfloat16, kind="ExternalInput")
    outd = nc.dram_tensor("outd", [128, W * 128], dt.float32,
                          kind="ExternalOutput")

    # group -> section table
    g2sec = []
    for sec, ng in enumerate(splits):
        g2sec.extend([sec] * ng)
    assert len(g2sec) == NG

    with tile.TileContext(nc) as tc:
        with tc.tile_pool(name="const", bufs=1) as cp, \
             tc.tile_pool(name="gath", bufs=2) as gp, \
             tc.tile_pool(name="ea", bufs=2) as ep, \
             tc.tile_pool(name="msg", bufs=2) as mp, \
             tc.tile_pool(name="mem", bufs=2) as memp, \
             tc.tile_pool(name="small", bufs=3) as sp, \
             tc.tile_pool(name="stg", bufs=2) as stp, \
             tc.tile_pool(name="pmp", bufs=2, space="PSUM") as pmp, \
             tc.tile_pool(name="accp", bufs=2, space="PSUM") as accp:
            wa_sb = cp.tile([H, H], dt.bfloat16)
            wb_sb = cp.tile([H, H], dt.bfloat16)
            wc_sb = cp.tile([INV + 1, H], dt.bfloat16)
            w2_sb = cp.tile([128, H], dt.bfloat16)
            b2_sb = cp.tile([128, 1], dt.float32)
            io_sb = cp.tile([128, 128], dt.bfloat16)
            idx_sb = cp.tile([128, NG * ICOL_G], dt.int16)
            rl_sb = cp.tile([128, NG * GS], dt.bfloat16)
            nc.sync.dma_start(out=wa_sb[:], in_=wa[:, :])
            nc.sync.dma_start(out=wb_sb[:], in_=wb[:, :])
            nc.sync.dma_start(out=wc_sb[:], in_=wc[:, :])
            nc.sync.dma_start(out=w2_sb[:], in_=w2b[:, :])
            nc.sync.dma_start(out=b2_sb[:], in_=b2t[:, :])
            nc.sync.dma_start(out=io_sb[:], in_=iot[:, :])
            nc.sync.dma_start(out=idx_sb[:], in_=idxt[:, :])
            nc.sync.dma_start(out=rl_sb[:], in_=rlt[:, :])

            for g in range(NG):
                gts = gp.tile([128, 1, SLOT_G], dt.bfloat16, tag="gts")
                gtr = gp.tile([128, 1, SLOT_G], dt.bfloat16, tag="gtr")
                c0 = g * ICOL_G
                ch = ICOL_G // 2
                nc.gpsimd.dma_gather(
                    gts[:], xsts[g2sec[g]][:, :], idx_sb[:, c0:c0 + ch],
                    SLOT_G, SLOT_G, H, transpose=True, single_packet=False)
                nc.gpsimd.dma_gather(
                    gtr[:], xrt[:, :], idx_sb[:, c0 + ch:c0 + ICOL_G],
                    SLOT_G, SLOT_G, H, transpose=True, single_packet=False)
                ea_sb = ep.tile([INV + 1, SLOT_G], dt.bfloat16, tag="ea")
                nc.sync.dma_start(
                    out=ea_sb[:], in_=eat[:, g * SLOT_G:(g + 1) * SLOT_G])

                msg = mp.tile([128, GS, H], dt.bfloat16, tag="msg")
                tt = mp.tile([128, GS, H], dt.bfloat16, tag="tt")
                red = sp.tile([128, GS], dt.float32, tag="red")
                gate = sp.tile([128, GS], dt.bfloat16, tag="gate")
                mm = memp.tile([128, GS, 128], dt.bfloat16, tag="mm")

                for b in range(GS // NB):
                    # pad to 8 slices = 4KB/partition = exactly 2 PSUM banks
                    # so double-buffered tiles never share a bank (PE-write +
                    # ScalarE-read of one bank is a fatal HW collision).
                    pm = pmp.tile([128, 8, H], dt.float32, tag="pm")
                    for jj in range(NB):
                        j = b * NB + jj
                        js = slice(j * 128, (j + 1) * 128)
                        nc.tensor.matmul(
                            out=pm[:, jj, :], lhsT=gts[:, 0, js], rhs=wa_sb[:],
                            start=True, stop=False)
                        nc.tensor.matmul(
                            out=pm[:, jj, :], lhsT=gtr[:, 0, js], rhs=wb_sb[:],
                            start=False, stop=False)
                        nc.tensor.matmul(
                            out=pm[:, jj, :], lhsT=ea_sb[:, js], rhs=wc_sb[:],
                            start=False, stop=True)
                    nc.scalar.activation(
                        out=msg[:, b * NB:(b + 1) * NB, :], in_=pm[:, :NB, :],
                        func=mybir.ActivationFunctionType.Silu)

                nc.vector.tensor_tensor(
                    out=tt[:, :, :], in0=msg[:, :, :],
                    in1=w2_sb[:, None, :].to_broadcast([128, GS, H]),
                    op=mybir.AluOpType.mult)
                nc.vector.tensor_reduce(
                    out=red[:], in_=tt[:, :, :],
                    axis=mybir.AxisListType.X, op=mybir.AluOpType.add)
                nc.scalar.activation(
                    out=gate[:], in_=red[:],
                    func=mybir.ActivationFunctionType.Sigmoid, bias=b2_sb[:, :])
                nc.vector.tensor_tensor(
                    out=mm[:, :, :],
                    in0=rl_sb[:, g * GS:(g + 1) * GS, None]
                        .to_broadcast([128, GS, 128]),
                    in1=io_sb[:, None, :].to_broadcast([128, GS, 128]),
                    op=mybir.AluOpType.is_equal)
                nc.vector.tensor_tensor(
                    out=mm[:, :, :], in0=mm[:, :, :],
                    in1=gate[:, :, None].to_broadcast([128, GS, 128]),
                    op=mybir.AluOpType.mult)

                # 8 slices = 2 PSUM banks exactly (bank-collision avoidance)
                acc = accp.tile([128, 8, 128], dt.float32, tag="acc")
                for w in range(G):
                    for s in range(S):
                        j = w * S + s
                        nc.tensor.matmul(
                            out=acc[:, w, :], lhsT=mm[:, j, :],
                            rhs=msg[:, j, :],
                            start=(s == 0), stop=(s == S - 1))
                stage = stp.tile([128, G, 128], dt.float32, tag="stage")
                nc.vector.tensor_copy(out=stage[:], in_=acc[:, :G, :])
                nc.sync.dma_start(
                    out=outd[:, g * G * 128:(g + 1) * G * 128],
                    in_=stage[:])
    nc.compile()
    _prog_cache[key] = nc
    return nc


BN_EPS = 1e-5


def _host_prep(x_send, x_rec, index, edge_attr, bn_gamma, bn_beta, bn_mean,
               bn_var, W1, b1, W2, b2):
    s_all = np.asarray(index[0], dtype=np.int64)
    r_all = np.asarray(index[1], dtype=np.int64)
    ea_all = np.asarray(edge_attr, dtype=np.float32)

    scale = np.asarray(bn_gamma) / np.sqrt(np.asarray(bn_var) + BN_EPS)
    shift = np.asarray(bn_beta) - np.asarray(bn_mean) * scale
    W1f = (np.asarray(W1) * scale[:, None]).astype(np.float32)
    b1f = (np.asarray(b1) + shift @ np.asarray(W1)).astype(np.float32)

    xs_bf = np.asarray(x_send, dtype=np.float32).astype(bfloat16)
    xr_bf = np.asarray(x_rec, dtype=np.float32).astype(bfloat16)
    ea_bf = ea_all.astype(bfloat16)

    wa_v = W1f[:H].astype(bfloat16)
    wb_v = W1f[H:2 * H].astype(bfloat16)
    wc_v = np.concatenate([W1f[2 * H:], b1f[None, :]], axis=0).astype(bfloat16)
    w2_v = np.broadcast_to(
        np.asarray(W2, dtype=np.float32).reshape(1, H), (128, H)
    ).astype(bfloat16)
    b2_v = np.full((128, 1), float(np.asarray(b2).reshape(-1)[0]),
                   dtype=np.float32)
    io_v = np.broadcast_to(
        np.arange(128, dtype=np.float32), (128, 128)).astype(bfloat16)

    # per-core window stats to pick S
    S = 6
    for k in range(NCORES):
        rk = r_all[(r_all // NLOC) == k] - k * NLOC
        cnt = np.bincount(rk // 128, minlength=W)
        S = max(S, int((cnt.max() + 127) // 128))

    GS = G * S
    SLOT_G = GS * 128
    SLOTS = NG * SLOT_G
    ICOL_G = 2 * SLOT_G // 16

    def wrap_idx(flat):
        # [SLOT_G] int -> [128, SLOT_G//16] int16, wrapped + replicated
        a = flat.reshape(SLOT_G // 16, 16).T.astype(np.int16)  # [16, cols]
        return np.tile(a, (8, 1))

    # per-core slot arrays (first pass), then a section split valid for ALL
    # cores (SPMD shares one program), then per-core tables.
    core_data = []
    for k in range(NCORES):
        m = (r_all // NLOC) == k
        sk = s_all[m]
        rk = r_all[m] - k * NLOC
        eak = ea_bf[m]
        o = np.argsort(rk, kind="stable")
        sk, rk, eak = sk[o], rk[o], eak[o]
        w = rk // 128
        cnt = np.bincount(w, minlength=W)
        start = np.zeros(W + 1, dtype=np.int64)
        np.cumsum(cnt, out=start[1:])
        iw = np.arange(rk.size) - start[w]          # index within window
        slot = (w * S + iw // 128) * 128 + (iw % 128)

        sidx = np.zeros(SLOTS, dtype=np.int64)
        sidx[slot] = sk
        rloc = np.full(SLOTS, -1.0, dtype=np.float32)
        rloc[slot] = (rk - w * 128).astype(np.float32)

        eat_v = np.zeros((INV + 1, SLOTS), dtype=bfloat16)
        eat_v[:INV, slot] = eak.T
        eat_v[INV, slot] = np.asarray(1.0, dtype=bfloat16)

        rk_slots = np.zeros(SLOTS, dtype=np.int64)
        rk_slots[slot] = rk
        core_data.append((sidx, rloc, eat_v, rk_slots))

    def fits(counts):
        bounds = np.cumsum([0] + counts)
        for sidx, _, _, _ in core_data:
            for si in range(len(counts)):
                seg = slice(bounds[si] * SLOT_G, bounds[si + 1] * SLOT_G)
                if np.unique(sidx[seg]).size > 32768:
                    return False
        return True

    sec_counts = [NG // 2, NG - NG // 2]
    while not fits(sec_counts):
        sec_counts = [h for c in sec_counts
                      for h in ((c + 1) // 2, c // 2) if h > 0]
        assert len(sec_counts) <= NG

    in_maps = []
    for k in range(NCORES):
        sidx, rloc, eat_v, rk_slots = core_data[k]
        bounds = np.cumsum([0] + sec_counts)
        xst_list = []
        sloc = np.empty(SLOTS, dtype=np.int64)
        for si in range(len(sec_counts)):
            g0, g1 = bounds[si], bounds[si + 1]
            seg = slice(g0 * SLOT_G, g1 * SLOT_G)
            uniq, inv = np.unique(sidx[seg], return_inverse=True)
            tbl = np.zeros((32768, H), dtype=bfloat16)
            tbl[:uniq.size] = xs_bf[uniq]
            xst_list.append(tbl)
            sloc[seg] = inv

        xr_tbl = np.zeros((XR_ROWS, H), dtype=bfloat16)
        xr_tbl[:NLOC] = xr_bf[k * NLOC:(k + 1) * NLOC]

        idx_v = np.zeros((128, NG * ICOL_G), dtype=np.int16)
        for g in range(NG):
            c0 = g * ICOL_G
            ch = ICOL_G // 2
            seg = slice(g * SLOT_G, (g + 1) * SLOT_G)
            idx_v[:, c0:c0 + ch] = wrap_idx(sloc[seg])
            idx_v[:, c0 + ch:c0 + ICOL_G] = wrap_idx(rk_slots[seg])

        rl_v = rloc.reshape(NG * GS, 128).T.astype(bfloat16)

        im = {"xrt": xr_tbl, "idxt": idx_v, "eat": eat_v, "rlt": rl_v,
              "wa": wa_v, "wb": wb_v, "wc": wc_v, "w2b": w2_v, "b2t": b2_v,
              "iot": io_v}
        for si, tbl in enumerate(xst_list):
            im[f"xst{si}"] = tbl
        in_maps.append(im)
    return in_maps, S, tuple(sec_counts)


def kernel(**inputs) -> np.ndarray:
    in_maps, S, splits = _host_prep(**inputs)
    nc = _build(S, splits)
    res = run_bass_kernel_spmd(nc, in_maps, core_ids=list(range(NCORES)))
    outs = []
    for k in range(NCORES):
        o = res.results[k]["outd"].reshape(128, W, 128)
        outs.append(o.transpose(1, 0, 2).reshape(W * 128, H)[:NLOC])
    return np.concatenate(outs, axis=0).astype(np.float32)


# revision 22
# speedup vs baseline: 3.0217x; 1.0120x over previous
"""ETNN messager layer on 8 Trainium2 NeuronCores — v2.

Receiver-sharded, window-batched. Core k owns receivers [k*12500, (k+1)*12500),
split into 98 windows of 128 receivers; each window gets S=6 subtiles of 128
edge slots (704+ edges/window never observed; S adapts upward if needed).
Per group of 7 windows: two transposing dma_gathers (bf16) pull x_send /
x_rec rows into [H, edges] layout, bf16 matmuls with BN-folded W1 produce
messages, gate = sigmoid(msg @ W2 + b2), and a gated one-hot membership
matmul segment-sums the window in PSUM. Output is written with plain
sequential DMA — no indirect scatter. Sender tables are compacted per
section on host so gather indices fit int16.
"""

import numpy as np
from ml_dtypes import bfloat16

import concourse.tile as tile
from concourse import bacc, bass, mybir
from concourse.bass_utils import run_bass_kernel_spmd

N = 100000
E = 500000
H = 128
INV = 16
NCORES = 8
NLOC = N // NCORES            # 12500 receivers per core
W = 98                        # 128-receiver windows per core (98*128 = 12544)
G = 7                         # windows per group
NG = W // G                   # 14 groups
XR_ROWS = W * 128             # receiver table rows (< 32768 so int16 works)

_prog_cache = {}


def _build(S: int, splits: tuple):
    """S = subtiles per window; splits = per-section group counts for the
    compacted sender tables (e.g. (7, 7))."""
    key = (S, splits)
    if key in _prog_cache:
        return _prog_cache[key]

    GS = G * S                # subtiles per group
    SLOT_G = GS * 128         # edge slots per group
    SLOTS = NG * SLOT_G       # edge slots per core
    ICOL_G = 2 * SLOT_G // 16  # idx columns per group (xs then xr)
    NB = 6                    # subtiles per silu batch
    assert GS % NB == 0

    nc = bacc.Bacc("TRN2", target_bir_lowering=False, debug=False)
    dt = mybir.dt
    xsts = [
        nc.dram_tensor(f"xst{i}", [32768, H], dt.bfloat16, kind="ExternalInput")
        for i in range(len(splits))
    ]
    xrt = nc.dram_tensor("xrt", [XR_ROWS, H], dt.bfloat16, kind="ExternalInput")
    idxt = nc.dram_tensor("idxt", [128, NG * ICOL_G], dt.int16,
                          kind="ExternalInput")
    eat = nc.dram_tensor("eat", [INV + 1, SLOTS], dt.bfloat16,
                         kind="ExternalInput")
    rlt = nc.dram_tensor("rlt", [128, NG * GS], dt.bfloat16,
                         kind="ExternalInput")
    wa = nc.dram_tensor("wa", [H, H], dt.bfloat16, kind="ExternalInput")
    wb = nc.dram_tensor("wb", [H, H], dt.bfloat16, kind="ExternalInput")
    wc = nc.dram_tensor("wc", [INV + 1, H], dt.bfloat16, kind="ExternalInput")
    w2b = nc.dram_tensor("w2b", [128, H], dt.bfloat16, kind="ExternalInput")
    b2t = nc.dram_tensor("b2t", [128, 1], dt.float32, kind="ExternalInput")
    iot = nc.dram_tensor("iot", [128, 128], dt.bfloat16, kind="ExternalInput")
    outd = nc.dram_tensor("outd", [128, W * 128], dt.float32,
                          kind="ExternalOutput")

    # group -> section table
    g2sec = []
    for sec, ng in enumerate(splits):
        g2sec.extend([sec] * ng)
    assert len(g2sec) == NG

    with tile.TileContext(nc) as tc:
        with tc.tile_pool(name="const", bufs=1) as cp, \
             tc.tile_pool(name="gath", bufs=2) as gp, \
             tc.tile_pool(name="ea", bufs=2) as ep, \
             tc.tile_pool(name="msg", bufs=2) as mp, \
             tc.tile_pool(name="mem", bufs=2) as memp, \
             tc.tile_pool(name="small", bufs=3) as sp, \
             tc.tile_pool(name="stg", bufs=2) as stp, \
             tc.tile_pool(name="pmp", bufs=2, space="PSUM") as pmp, \
             tc.tile_pool(name="accp", bufs=2, space="PSUM") as accp:
            wa_sb = cp.tile([H, H], dt.bfloat16)
            wb_sb = cp.tile([H, H], dt.bfloat16)
            wc_sb = cp.tile([INV + 1, H], dt.bfloat16)
            w2_sb = cp.tile([128, H], dt.bfloat16)
            b2_sb = cp.tile([128, 1], dt.float32)
            io_sb = cp.tile([128, 128], dt.bfloat16)
            idx_sb = cp.tile([128, NG * ICOL_G], dt.int16)
            rl_sb = cp.tile([128, NG * GS], dt.bfloat16)
            nc.sync.dma_start(out=wa_sb[:], in_=wa[:, :])
            nc.sync.dma_start(out=wb_sb[:], in_=wb[:, :])
            nc.sync.dma_start(out=wc_sb[:], in_=wc[:, :])
            nc.sync.dma_start(out=w2_sb[:], in_=w2b[:, :])
            nc.sync.dma_start(out=b2_sb[:], in_=b2t[:, :])
            nc.sync.dma_start(out=io_sb[:], in_=iot[:, :])
            nc.sync.dma_start(out=idx_sb[:], in_=idxt[:, :])
            nc.sync.dma_start(out=rl_sb[:], in_=rlt[:, :])

            for g in range(NG):
                gts = gp.tile([128, 1, SLOT_G], dt.bfloat16, tag="gts")
                gtr = gp.tile([128, 1, SLOT_G], dt.bfloat16, tag="gtr")
                c0 = g * ICOL_G
                ch = ICOL_G // 2
                nc.gpsimd.dma_gather(
                    gts[:], xsts[g2sec[g]][:, :], idx_sb[:, c0:c0 + ch],
                    SLOT_G, SLOT_G, H, transpose=True, single_packet=False)
                nc.gpsimd.dma_gather(
                    gtr[:], xrt[:, :], idx_sb[:, c0 + ch:c0 + ICOL_G],
                    SLOT_G, SLOT_G, H, transpose=True, single_packet=False)
                ea_sb = ep.tile([INV + 1, SLOT_G], dt.bfloat16, tag="ea")
                nc.sync.dma_start(
                    out=ea_sb[:], in_=eat[:, g * SLOT_G:(g + 1) * SLOT_G])

                msg = mp.tile([128, GS, H], dt.bfloat16, tag="msg")
                tt = mp.tile([128, GS, H], dt.bfloat16, tag="tt")
                red = sp.tile([128, GS], dt.float32, tag="red")
                gate = sp.tile([128, GS], dt.bfloat16, tag="gate")
                mm = memp.tile([128, GS, 128], dt.bfloat16, tag="mm")

                for b in range(GS // NB):
                    # pad to 8 slices = 4KB/partition = exactly 2 PSUM banks
                    # so double-buffered tiles never share a bank (PE-write +
                    # ScalarE-read of one bank is a fatal HW collision).
                    pm = pmp.tile([128, 8, H], dt.float32, tag="pm")
                    for jj in range(NB):
                        j = b * NB + jj
                        js = slice(j * 128, (j + 1) * 128)
                        nc.tensor.matmul(
                            out=pm[:, jj, :], lhsT=gts[:, 0, js], rhs=wa_sb[:],
                            start=True, stop=False)
                        nc.tensor.matmul(
                            out=pm[:, jj, :], lhsT=gtr[:, 0, js], rhs=wb_sb[:],
                            start=False, stop=False)
                        nc.tensor.matmul(
                            out=pm[:, jj, :], lhsT=ea_sb[:, js], rhs=wc_sb[:],
                            start=False, stop=True)
                    nc.scalar.activation(
                        out=msg[:, b * NB:(b + 1) * NB, :], in_=pm[:, :NB, :],
                        func=mybir.ActivationFunctionType.Silu)

                nc.vector.tensor_tensor(
                    out=tt[:, :, :], in0=msg[:, :, :],
                    in1=w2_sb[:, None, :].to_broadcast([128, GS, H]),
                    op=mybir.AluOpType.mult)
                nc.vector.tensor_reduce(
                    out=red[:], in_=tt[:, :, :],
                    axis=mybir.AxisListType.X, op=mybir.AluOpType.add)
                nc.scalar.activation(
                    out=gate[:], in_=red[:],
                    func=mybir.ActivationFunctionType.Sigmoid, bias=b2_sb[:, :])
                nc.vector.tensor_tensor(
                    out=mm[:, :, :],
                    in0=rl_sb[:, g * GS:(g + 1) * GS, None]
                        .to_broadcast([128, GS, 128]),
                    in1=io_sb[:, None, :].to_broadcast([128, GS, 128]),
                    op=mybir.AluOpType.is_equal)
                nc.vector.tensor_tensor(
                    out=mm[:, :, :], in0=mm[:, :, :],
                    in1=gate[:, :, None].to_broadcast([128, GS, 128]),
                    op=mybir.AluOpType.mult)

                # 8 slices = 2 PSUM banks exactly (bank-collision avoidance)
                acc = accp.tile([128, 8, 128], dt.float32, tag="acc")
                for w in range(G):
                    for s in range(S):
                        j = w * S + s
                        nc.tensor.matmul(
                            out=acc[:, w, :], lhsT=mm[:, j, :],
                            rhs=msg[:, j, :],
                            start=(s == 0), stop=(s == S - 1))
                stage = stp.tile([128, G, 128], dt.float32, tag="stage")
                nc.vector.tensor_copy(out=stage[:], in_=acc[:, :G, :])
                nc.sync.dma_start(
                    out=outd[:, g * G * 128:(g + 1) * G * 128],
                    in_=stage[:])
    nc.compile()
    _prog_cache[key] = nc
    return nc


BN_EPS = 1e-5


def _host_prep(x_send, x_rec, index, edge_attr, bn_gamma, bn_beta, bn_mean,
               bn_var, W1, b1, W2, b2):
    s_all = np.asarray(index[0], dtype=np.int64)
    r_all = np.asarray(index[1], dtype=np.int64)
    ea_all = np.asarray(edge_attr, dtype=np.float32)

    scale = np.asarray(bn_gamma) / np.sqrt(np.asarray(bn_var) + BN_EPS)
    shift = np.asarray(bn_beta) - np.asarray(bn_mean) * scale
    W1f = (np.asarray(W1) * scale[:, None]).astype(np.float32)
    b1f = (np.asarray(b1) + shift @ np.asarray(W1)).astype(np.float32)

    xs_bf = np.asarray(x_send, dtype=np.float32).astype(bfloat16)
    xr_bf = np.asarray(x_rec, dtype=np.float32).astype(bfloat16)
    ea_bf = ea_all.astype(bfloat16)

    wa_v = W1f[:H].astype(bfloat16)
    wb_v = W1f[H:2 * H].astype(bfloat16)
    wc_v = np.concatenate([W1f[2 * H:], b1f[None, :]], axis=0).astype(bfloat16)
    w2_v = np.broadcast_to(
        np.asarray(W2, dtype=np.float32).reshape(1, H), (128, H)
    ).astype(bfloat16)
    b2_v = np.full((128, 1), float(np.asarray(b2).reshape(-1)[0]),
                   dtype=np.float32)
    io_v = np.broadcast_to(
        np.arange(128, dtype=np.float32), (128, 128)).astype(bfloat16)

    # per-core window stats to pick S
    S = 6
    for k in range(NCORES):
        rk = r_all[(r_all // NLOC) == k] - k * NLOC
        cnt = np.bincount(rk // 128, minlength=W)
        S = max(S, int((cnt.max() + 127) // 128))

    GS = G * S
    SLOT_G = GS * 128
    SLOTS = NG * SLOT_G
    ICOL_G = 2 * SLOT_G // 16

    def wrap_idx(flat):
        # [SLOT_G] int -> [128, SLOT_G//16] int16, wrapped + replicated
        a = flat.reshape(SLOT_G // 16, 16).T.astype(np.int16)  # [16, cols]
        return np.tile(a, (8, 1))

    # per-core slot arrays (first pass), then a section split valid for ALL
    # cores (SPMD shares one program), then per-core tables.
    core_data = []
    for k in range(NCORES):
        m = (r_all // NLOC) == k
        sk = s_all[m]
        rk = r_all[m] - k * NLOC
        eak = ea_bf[m]
        o = np.argsort(rk, kind="stable")
        sk, rk, eak = sk[o], rk[o], eak[o]
        w = rk // 128
        cnt = np.bincount(w, minlength=W)
        start = np.zeros(W + 1, dtype=np.int64)
        np.cumsum(cnt, out=start[1:])
        iw = np.arange(rk.size) - start[w]          # index within window
        slot = (w * S + iw // 128) * 128 + (iw % 128)

        sidx = np.zeros(SLOTS, dtype=np.int64)
        sidx[slot] = sk
        rloc = np.full(SLOTS, -1.0, dtype=np.float32)
        rloc[slot] = (rk - w * 128).astype(np.float32)

        eat_v = np.zeros((INV + 1, SLOTS), dtype=bfloat16)
        eat_v[:INV, slot] = eak.T
        eat_v[INV, slot] = np.asarray(1.0, dtype=bfloat16)

        rk_slots = np.zeros(SLOTS, dtype=np.int64)
        rk_slots[slot] = rk
        core_data.append((sidx, rloc, eat_v, rk_slots))

    def fits(counts):
        bounds = np.cumsum([0] + counts)
        for sidx, _, _, _ in core_data:
            for si in range(len(counts)):
                seg = slice(bounds[si] * SLOT_G, bounds[si + 1] * SLOT_G)
                if np.unique(sidx[seg]).size > 32768:
                    return False
        return True

    sec_counts = [NG // 2, NG - NG // 2]
    while not fits(sec_counts):
        sec_counts = [h for c in sec_counts
                      for h in ((c + 1) // 2, c // 2) if h > 0]
        assert len(sec_counts) <= NG

    in_maps = []
    for k in range(NCORES):
        sidx, rloc, eat_v, rk_slots = core_data[k]
        bounds = np.cumsum([0] + sec_counts)
        xst_list = []
        sloc = np.empty(SLOTS, dtype=np.int64)
        for si in range(len(sec_counts)):
            g0, g1 = bounds[si], bounds[si + 1]
            seg = slice(g0 * SLOT_G, g1 * SLOT_G)
            uniq, inv = np.unique(sidx[seg], return_inverse=True)
            tbl = np.zeros((32768, H), dtype=bfloat16)
            tbl[:uniq.size] = xs_bf[uniq]
            xst_list.append(tbl)
            sloc[seg] = inv

        xr_tbl = np.zeros((XR_ROWS, H), dtype=bfloat16)
        xr_tbl[:NLOC] = xr_bf[k * NLOC:(k + 1) * NLOC]

        idx_v = np.zeros((128, NG * ICOL_G), dtype=np.int16)
        for g in range(NG):
            c0 = g * ICOL_G
            ch = ICOL_G // 2
            seg = slice(g * SLOT_G, (g + 1) * SLOT_G)
            idx_v[:, c0:c0 + ch] = wrap_idx(sloc[seg])
            idx_v[:, c0 + ch:c0 + ICOL_G] = wrap_idx(rk_slots[seg])

        rl_v = rloc.reshape(NG * GS, 128).T.astype(bfloat16)

        im = {"xrt": xr_tbl, "idxt": idx_v, "eat": eat_v, "rlt": rl_v,
              "wa": wa_v, "wb": wb_v, "wc": wc_v, "w2b": w2_v, "b2t": b2_v,
              "iot": io_v}
        for si, tbl in enumerate(xst_list):
            im[f"xst{si}"] = tbl
        in_maps.append(im)
    return in_maps, S, tuple(sec_counts)


def kernel(**inputs) -> np.ndarray:
    in_maps, S, splits = _host_prep(**inputs)
    nc = _build(S, splits)
    res = run_bass_kernel_spmd(nc, in_maps, core_ids=list(range(NCORES)))
    outs = []
    for k in range(NCORES):
        o = res.results[k]["outd"].reshape(128, W, 128)
        outs.append(o.transpose(1, 0, 2).reshape(W * 128, H)[:NLOC])
    return np.concatenate(outs, axis=0).astype(np.float32)


# revision 24
# speedup vs baseline: 5.4800x; 1.8135x over previous
"""ETNN messager layer on 8 Trainium2 NeuronCores — v2.

Receiver-sharded, window-batched. Core k owns receivers [k*12500, (k+1)*12500),
split into 98 windows of 128 receivers; each window gets S=6 subtiles of 128
edge slots (704+ edges/window never observed; S adapts upward if needed).
Per group of 7 windows: two transposing dma_gathers (bf16) pull x_send /
x_rec rows into [H, edges] layout, bf16 matmuls with BN-folded W1 produce
messages, gate = sigmoid(msg @ W2 + b2), and a gated one-hot membership
matmul segment-sums the window in PSUM. Output is written with plain
sequential DMA — no indirect scatter. Sender tables are compacted per
section on host so gather indices fit int16.
"""

import numpy as np
from ml_dtypes import bfloat16

import concourse.tile as tile
from concourse import bacc, bass, mybir
from concourse.bass_utils import run_bass_kernel_spmd
from concourse.masks import make_identity

N = 100000
E = 500000
H = 128
INV = 16
NCORES = 8
NLOC = N // NCORES            # 12500 receivers per core
W = 98                        # 128-receiver windows per core (98*128 = 12544)
G = 7                         # windows per group
NG = W // G                   # 14 groups
XR_ROWS = W * 128             # receiver table rows (< 32768 so int16 works)

_prog_cache = {}


def _build(S: int, splits: tuple):
    """S = subtiles per window; splits = per-section group counts for the
    compacted sender tables (e.g. (7, 7))."""
    key = (S, splits)
    if key in _prog_cache:
        return _prog_cache[key]

    GS = G * S                # subtiles per group
    SLOT_G = GS * 128         # edge slots per group
    SLOTS = NG * SLOT_G       # edge slots per core
    ICOL_G = SLOT_G // 16     # idx columns per group (xs only; xr is dense)
    NB = 6                    # subtiles per silu batch
    assert GS % NB == 0

    nc = bacc.Bacc("TRN2", target_bir_lowering=False, debug=False)
    dt = mybir.dt
    xsts = [
        nc.dram_tensor(f"xst{i}", [32768, H], dt.bfloat16, kind="ExternalInput")
        for i in range(len(splits))
    ]
    xrt = nc.dram_tensor("xrt", [W, 128, H], dt.bfloat16, kind="ExternalInput")
    idxt = nc.dram_tensor("idxt", [128, NG * ICOL_G], dt.int16,
                          kind="ExternalInput")
    eat = nc.dram_tensor("eat", [INV + 1, SLOTS], dt.bfloat16,
                         kind="ExternalInput")
    rlt = nc.dram_tensor("rlt", [128, NG * GS], dt.bfloat16,
                         kind="ExternalInput")
    rltT = nc.dram_tensor("rltT", [128, NG * GS, 128], dt.bfloat16,
                          kind="ExternalInput")
    ioc = nc.dram_tensor("ioc", [128, 1], dt.bfloat16, kind="ExternalInput")
    wa = nc.dram_tensor("wa", [H, H], dt.bfloat16, kind="ExternalInput")
    wb = nc.dram_tensor("wb", [H, H], dt.bfloat16, kind="ExternalInput")
    wc = nc.dram_tensor("wc", [INV + 1, H], dt.bfloat16, kind="ExternalInput")
    w2b = nc.dram_tensor("w2b", [128, H], dt.bfloat16, kind="ExternalInput")
    b2t = nc.dram_tensor("b2t", [128, 1], dt.float32, kind="ExternalInput")
    iot = nc.dram_tensor("iot", [128, 128], dt.bfloat16, kind="ExternalInput")
    outd = nc.dram_tensor("outd", [128, W * 128], dt.float32,
                          kind="ExternalOutput")

    # group -> section table
    g2sec = []
    for sec, ng in enumerate(splits):
        g2sec.extend([sec] * ng)
    assert len(g2sec) == NG

    with tile.TileContext(nc) as tc:
        with tc.tile_pool(name="const", bufs=1) as cp, \
             tc.tile_pool(name="gath", bufs=2) as gp, \
             tc.tile_pool(name="ea", bufs=2) as ep, \
             tc.tile_pool(name="msg", bufs=2) as mp, \
             tc.tile_pool(name="mem", bufs=2) as memp, \
             tc.tile_pool(name="small", bufs=3) as sp, \
             tc.tile_pool(name="stg", bufs=2) as stp, \
             tc.tile_pool(name="pmp", bufs=2, space="PSUM") as pmp, \
             tc.tile_pool(name="accp", bufs=2, space="PSUM") as accp:
            wa_sb = cp.tile([H, H], dt.bfloat16)
            wb_sb = cp.tile([H, H], dt.bfloat16)
            wc_sb = cp.tile([INV + 1, H], dt.bfloat16)
            w2_sb = cp.tile([128, H], dt.bfloat16)
            b2_sb = cp.tile([128, 1], dt.float32)
            io_sb = cp.tile([128, 128], dt.bfloat16)
            ioc_sb = cp.tile([128, 1], dt.bfloat16)
            ident = cp.tile([128, 128], dt.bfloat16)
            make_identity(nc, ident[:])
            idx_sb = cp.tile([128, NG * ICOL_G], dt.int16)
            rl_sb = cp.tile([128, NG * GS], dt.bfloat16)
            nc.sync.dma_start(out=wa_sb[:], in_=wa[:, :])
            nc.sync.dma_start(out=wb_sb[:], in_=wb[:, :])
            nc.sync.dma_start(out=wc_sb[:], in_=wc[:, :])
            nc.sync.dma_start(out=w2_sb[:], in_=w2b[:, :])
            nc.sync.dma_start(out=b2_sb[:], in_=b2t[:, :])
            nc.sync.dma_start(out=io_sb[:], in_=iot[:, :])
            nc.sync.dma_start(out=ioc_sb[:], in_=ioc[:, :])
            nc.sync.dma_start(out=idx_sb[:], in_=idxt[:, :])
            nc.sync.dma_start(out=rl_sb[:], in_=rlt[:, :])

            for g in range(NG):
                gts = gp.tile([128, 1, SLOT_G], dt.bfloat16, tag="gts")
                c0 = g * ICOL_G
                nc.gpsimd.dma_gather(
                    gts[:], xsts[g2sec[g]][:, :], idx_sb[:, c0:c0 + ICOL_G],
                    SLOT_G, SLOT_G, H, transpose=True, single_packet=False)
                # dense natural load of this group's contiguous receivers,
                # PE-transposed on chip (DMA-transpose races the concurrent
                # transpose-gathers on the xbar and corrupts data)
                xrb = gp.tile([128, G, H], dt.bfloat16, tag="xrb")
                for w in range(G):
                    nc.sync.dma_start(
                        out=xrb[:, w, :], in_=xrt[g * G + w, :, :])
                tps = pmp.tile([128, 16, H], dt.bfloat16, tag="pm")
                for w in range(G):
                    nc.tensor.transpose(
                        out=tps[:, w, :], in_=xrb[:, w, :], identity=ident[:])
                xrT = gp.tile([128, G, H], dt.bfloat16, tag="xrT")
                nc.vector.tensor_copy(out=xrT[:], in_=tps[:, :G, :])

                rlT = memp.tile([128, GS, 128], dt.bfloat16, tag="rlT")
                nc.sync.dma_start(
                    out=rlT[:],
                    in_=rltT[:, g * GS:(g + 1) * GS, :])
                ea_sb = ep.tile([INV + 1, SLOT_G], dt.bfloat16, tag="ea")
                nc.sync.dma_start(
                    out=ea_sb[:], in_=eat[:, g * SLOT_G:(g + 1) * SLOT_G])

                msg = mp.tile([128, GS, H], dt.bfloat16, tag="msg")
                tt = mp.tile([128, GS, H], dt.bfloat16, tag="tt")
                red = sp.tile([128, GS], dt.float32, tag="red")
                gate = sp.tile([128, GS], dt.bfloat16, tag="gate")
                mm = memp.tile([128, GS, 128], dt.bfloat16, tag="mm")
                mmT = memp.tile([128, GS, 128], dt.bfloat16, tag="mmT")
                yr_sb = sp.tile([128, G, H], dt.bfloat16, tag="yr")

                # yr_w = xr_w @ Wb for the 7 windows (dense, no gather)
                pmy = pmp.tile([128, 8, H], dt.float32, tag="pm")
                for w in range(G):
                    nc.tensor.matmul(
                        out=pmy[:, w, :],
                        lhsT=xrT[:, w, :], rhs=wb_sb[:],
                        start=True, stop=True)
                nc.vector.tensor_copy(out=yr_sb[:], in_=pmy[:, :G, :])
                # transposed membership: mmT[r, e] = (rloc[e] == r)
                nc.vector.tensor_tensor(
                    out=mmT[:, :, :],
                    in0=ioc_sb[:, :, None].to_broadcast([128, GS, 128]),
                    in1=rlT[:, :, :],
                    op=mybir.AluOpType.is_equal)

                for b in range(GS // NB):
                    # pad to 8 slices = 4KB/partition = exactly 2 PSUM banks
                    # so double-buffered tiles never share a bank (PE-write +
                    # ScalarE-read of one bank is a fatal HW collision).
                    pm = pmp.tile([128, 8, H], dt.float32, tag="pm")
                    for jj in range(NB):
                        j = b * NB + jj
                        w = j // S
                        js = slice(j * 128, (j + 1) * 128)
                        nc.tensor.matmul(
                            out=pm[:, jj, :], lhsT=gts[:, 0, js], rhs=wa_sb[:],
                            start=True, stop=False)
                        nc.tensor.matmul(
                            out=pm[:, jj, :], lhsT=mmT[:, j, :],
                            rhs=yr_sb[:, w, :],
                            start=False, stop=False)
                        nc.tensor.matmul(
                            out=pm[:, jj, :], lhsT=ea_sb[:, js], rhs=wc_sb[:],
                            start=False, stop=True)
                    nc.scalar.activation(
                        out=msg[:, b * NB:(b + 1) * NB, :], in_=pm[:, :NB, :],
                        func=mybir.ActivationFunctionType.Silu)

                nc.vector.tensor_tensor(
                    out=tt[:, :, :], in0=msg[:, :, :],
                    in1=w2_sb[:, None, :].to_broadcast([128, GS, H]),
                    op=mybir.AluOpType.mult)
                nc.vector.tensor_reduce(
                    out=red[:], in_=tt[:, :, :],
                    axis=mybir.AxisListType.X, op=mybir.AluOpType.add)
                nc.scalar.activation(
                    out=gate[:], in_=red[:],
                    func=mybir.ActivationFunctionType.Sigmoid, bias=b2_sb[:, :])
                nc.vector.tensor_tensor(
                    out=mm[:, :, :],
                    in0=rl_sb[:, g * GS:(g + 1) * GS, None]
                        .to_broadcast([128, GS, 128]),
                    in1=io_sb[:, None, :].to_broadcast([128, GS, 128]),
                    op=mybir.AluOpType.is_equal)
                nc.vector.tensor_tensor(
                    out=mm[:, :, :], in0=mm[:, :, :],
                    in1=gate[:, :, None].to_broadcast([128, GS, 128]),
                    op=mybir.AluOpType.mult)

                # 8 slices = 2 PSUM banks exactly (bank-collision avoidance)
                acc = accp.tile([128, 8, 128], dt.float32, tag="acc")
                for w in range(G):
                    for s in range(S):
                        j = w * S + s
                        nc.tensor.matmul(
                            out=acc[:, w, :], lhsT=mm[:, j, :],
                            rhs=msg[:, j, :],
                            start=(s == 0), stop=(s == S - 1))
                stage = stp.tile([128, G, 128], dt.float32, tag="stage")
                nc.vector.tensor_copy(out=stage[:], in_=acc[:, :G, :])
                nc.sync.dma_start(
                    out=outd[:, g * G * 128:(g + 1) * G * 128],
                    in_=stage[:])
    nc.compile()
    _prog_cache[key] = nc
    return nc


BN_EPS = 1e-5


def _host_prep(x_send, x_rec, index, edge_attr, bn_gamma, bn_beta, bn_mean,
               bn_var, W1, b1, W2, b2):
    s_all = np.asarray(index[0], dtype=np.int64)
    r_all = np.asarray(index[1], dtype=np.int64)
    ea_all = np.asarray(edge_attr, dtype=np.float32)

    scale = np.asarray(bn_gamma) / np.sqrt(np.asarray(bn_var) + BN_EPS)
    shift = np.asarray(bn_beta) - np.asarray(bn_mean) * scale
    W1f = (np.asarray(W1) * scale[:, None]).astype(np.float32)
    b1f = (np.asarray(b1) + shift @ np.asarray(W1)).astype(np.float32)

    xs_bf = np.asarray(x_send, dtype=np.float32).astype(bfloat16)
    xr_bf = np.asarray(x_rec, dtype=np.float32).astype(bfloat16)
    ea_bf = ea_all.astype(bfloat16)

    wa_v = W1f[:H].astype(bfloat16)
    wb_v = W1f[H:2 * H].astype(bfloat16)
    wc_v = np.concatenate([W1f[2 * H:], b1f[None, :]], axis=0).astype(bfloat16)
    w2_v = np.broadcast_to(
        np.asarray(W2, dtype=np.float32).reshape(1, H), (128, H)
    ).astype(bfloat16)
    b2_v = np.full((128, 1), float(np.asarray(b2).reshape(-1)[0]),
                   dtype=np.float32)
    io_v = np.broadcast_to(
        np.arange(128, dtype=np.float32), (128, 128)).astype(bfloat16)

    # per-core window stats to pick S
    S = 6
    for k in range(NCORES):
        rk = r_all[(r_all // NLOC) == k] - k * NLOC
        cnt = np.bincount(rk // 128, minlength=W)
        S = max(S, int((cnt.max() + 127) // 128))

    GS = G * S
    SLOT_G = GS * 128
    SLOTS = NG * SLOT_G
    ICOL_G = SLOT_G // 16

    def wrap_idx(flat):
        # [SLOT_G] int -> [128, SLOT_G//16] int16, wrapped + replicated
        a = flat.reshape(SLOT_G // 16, 16).T.astype(np.int16)  # [16, cols]
        return np.tile(a, (8, 1))

    # per-core slot arrays (first pass), then a section split valid for ALL
    # cores (SPMD shares one program), then per-core tables.
    core_data = []
    for k in range(NCORES):
        m = (r_all // NLOC) == k
        sk = s_all[m]
        rk = r_all[m] - k * NLOC
        eak = ea_bf[m]
        o = np.argsort(rk, kind="stable")
        sk, rk, eak = sk[o], rk[o], eak[o]
        w = rk // 128
        cnt = np.bincount(w, minlength=W)
        start = np.zeros(W + 1, dtype=np.int64)
        np.cumsum(cnt, out=start[1:])
        iw = np.arange(rk.size) - start[w]          # index within window
        slot = (w * S + iw // 128) * 128 + (iw % 128)

        sidx = np.zeros(SLOTS, dtype=np.int64)
        sidx[slot] = sk
        rloc = np.full(SLOTS, -1.0, dtype=np.float32)
        rloc[slot] = (rk - w * 128).astype(np.float32)

        eat_v = np.zeros((INV + 1, SLOTS), dtype=bfloat16)
        eat_v[:INV, slot] = eak.T
        eat_v[INV, slot] = np.asarray(1.0, dtype=bfloat16)

        rk_slots = np.zeros(SLOTS, dtype=np.int64)
        rk_slots[slot] = rk
        core_data.append((sidx, rloc, eat_v, rk_slots))

    def fits(counts):
        bounds = np.cumsum([0] + counts)
        for sidx, _, _, _ in core_data:
            for si in range(len(counts)):
                seg = slice(bounds[si] * SLOT_G, bounds[si + 1] * SLOT_G)
                if np.unique(sidx[seg]).size > 32768:
                    return False
        return True

    sec_counts = [NG // 2, NG - NG // 2]
    while not fits(sec_counts):
        sec_counts = [h for c in sec_counts
                      for h in ((c + 1) // 2, c // 2) if h > 0]
        assert len(sec_counts) <= NG

    in_maps = []
    for k in range(NCORES):
        sidx, rloc, eat_v, rk_slots = core_data[k]
        bounds = np.cumsum([0] + sec_counts)
        xst_list = []
        sloc = np.empty(SLOTS, dtype=np.int64)
        for si in range(len(sec_counts)):
            g0, g1 = bounds[si], bounds[si + 1]
            seg = slice(g0 * SLOT_G, g1 * SLOT_G)
            uniq, inv = np.unique(sidx[seg], return_inverse=True)
            tbl = np.zeros((32768, H), dtype=bfloat16)
            tbl[:uniq.size] = xs_bf[uniq]
            xst_list.append(tbl)
            sloc[seg] = inv

        xr_tbl = np.zeros((XR_ROWS, H), dtype=bfloat16)
        xr_tbl[:NLOC] = xr_bf[k * NLOC:(k + 1) * NLOC]
        xr_tbl = xr_tbl.reshape(W, 128, H)

        idx_v = np.zeros((128, NG * ICOL_G), dtype=np.int16)
        for g in range(NG):
            c0 = g * ICOL_G
            seg = slice(g * SLOT_G, (g + 1) * SLOT_G)
            idx_v[:, c0:c0 + ICOL_G] = wrap_idx(sloc[seg])

        rl_v = rloc.reshape(NG * GS, 128).T.astype(bfloat16)
        rlT_v = np.broadcast_to(
            rloc.astype(bfloat16)[None, :], (128, SLOTS)
        ).reshape(128, NG * GS, 128).copy()
        ioc_v = np.arange(128, dtype=np.float32).reshape(128, 1).astype(bfloat16)

        im = {"xrt": xr_tbl, "idxt": idx_v, "eat": eat_v, "rlt": rl_v,
              "rltT": rlT_v, "ioc": ioc_v,
              "wa": wa_v, "wb": wb_v, "wc": wc_v, "w2b": w2_v, "b2t": b2_v,
              "iot": io_v}
        for si, tbl in enumerate(xst_list):
            im[f"xst{si}"] = tbl
        in_maps.append(im)
    return in_maps, S, tuple(sec_counts)


def kernel(**inputs) -> np.ndarray:
    in_maps, S, splits = _host_prep(**inputs)
    nc = _build(S, splits)
    res = run_bass_kernel_spmd(nc, in_maps, core_ids=list(range(NCORES)))
    outs = []
    for k in range(NCORES):
        o = res.results[k]["outd"].reshape(128, W, 128)
        outs.append(o.transpose(1, 0, 2).reshape(W * 128, H)[:NLOC])
    return np.concatenate(outs, axis=0).astype(np.float32)
